# revision 1
# baseline (speedup 1.0000x reference)
"""Trainium2 Bass kernel for CoordPE + message-passing GNN.

Sharding: 8 cores = 2 batches x 4 query-chunks of 512 rows each.
Each core computes dist/W/rbf for (all 2048 keys j) x (its 512 queries i),
in [j=128 part, i free] tiles, then runs the 3 MP layers for its rows with
an AllGather of h across the 4 cores of its batch group between layers.
"""
import sys

import numpy as np

sys.path.insert(0, "/opt/trn_rl_repo")

B, L, D, K, R = 2, 2048, 128, 3, 16
NCORES = 8
CH = L // 4          # 512 queries per core
NJT = L // 128       # 16 j-tiles
FREE = NJT * CH      # 8192 = free extent of the [128, 8192] pair tensors
HF = FREE // 2       # 4096 half-extent for the RBF pipeline


def build_program(gamma, centers, n_rep=1, use_cc=True, num_devices=NCORES,
                  debug_outs=False, loop_reps=0):
    import contextlib

    import concourse.tile as tile
    from concourse import bacc, mybir

    AF = mybir.ActivationFunctionType
    ALU = mybir.AluOpType
    dt = mybir.dt
    f32 = dt.float32
    f16 = dt.float16
    f32r = dt.float32r

    gamma = float(gamma)
    centers = [float(c) for c in centers]

    nc = bacc.Bacc("TRN2", target_bir_lowering=False, debug=False,
                   num_devices=num_devices)

    def dram_in(name, shape):
        return nc.dram_tensor(name, shape, f32, kind="ExternalInput")

    cT_all = dram_in("cT_all", [4, L])        # coords^T (3 rows + zero pad)
    cT_i = dram_in("cT_i", [4, CH])
    nrmj_t = dram_in("nrmj_t", [128, NJT])    # |x_j|^2 tiled [j-in-tile, jt]
    nrmi_bc = dram_in("nrmi_bc", [128, CH])   # |x_i|^2 row, pre-broadcast
    h_atomT = dram_in("h_atomT", [D, CH])
    rbf_w = dram_in("rbf_w", [R, D])
    out_w = dram_in("out_w", [D, 2 * D])      # [din, (half, dout)]
    self_w = dram_in("self_w", [D, K * D])    # [din, (k, dout)]
    msg_w = dram_in("msg_w", [D, K * D])
    upd_w = dram_in("upd_w", [D, K * 2 * D])  # [din, (k, half, dout)]
    gc2 = dram_in("gc2", [128, R])            # -gamma*c_r^2 (rows replicated)
    biases = dram_in("biases", [128, 8])      # rbf_b,out_b,self x3,upd x3
    msgb_bc = dram_in("msgb_bc", [128, K * D])  # msg_b pre-broadcast rows
    redmask = nc.dram_tensor("redmask", [128, R * R], f16,
                             kind="ExternalInput")  # col r of block r = 1/L
    out_hT = nc.dram_tensor("out_hT", [D, CH], f32, kind="ExternalOutput")
    if debug_outs:
        dbg_rbf = nc.dram_tensor("dbg_rbf", [R, CH], f32,
                                 kind="ExternalOutput")
        dbg_h0 = nc.dram_tensor("dbg_h0", [D, CH], f32, kind="ExternalOutput")
        dbg_w = nc.dram_tensor("dbg_w", [128, FREE], f16,
                               kind="ExternalOutput")

    s_r = [2.0 * gamma * c for c in centers]

    with tile.TileContext(nc) as tc:
        with (
            tc.tile_pool(name="const", bufs=1) as cpool,
            tc.tile_pool(name="big", bufs=1) as bigpool,
            tc.tile_pool(name="arg", bufs=3) as argpool,
            tc.tile_pool(name="work", bufs=1) as work,
            tc.tile_pool(name="hmyp", bufs=2) as hmyp,
            tc.tile_pool(name="hfullp", bufs=1) as hfullp,
            tc.tile_pool(name="psA", bufs=2, space="PSUM") as psA,
            tc.tile_pool(name="psB", bufs=1, space="PSUM") as psB,
            tc.tile_pool(name="psC", bufs=2, space="PSUM") as psC,
            tc.tile_pool(name="dram", bufs=1, space="DRAM") as dpool,
        ):
            # ---- load constants/weights ----
            def load(handle, shape, tag):
                t = cpool.tile(shape, f32, tag=tag)
                nc.sync.dma_start(t[:], handle.ap())
                return t

            # coords live in arg-pool slots (lifetime ends before args start)
            c_all = argpool.tile([4, L], f32, tag="arg")
            nc.sync.dma_start(c_all[:], cT_all.ap())
            c_i = argpool.tile([4, CH], f32, tag="q")
            nc.sync.dma_start(c_i[:], cT_i.ap())
            nj = load(nrmj_t, [128, NJT], "nj")
            ni = load(nrmi_bc, [128, CH], "ni")
            hat = load(h_atomT, [D, CH], "hat")
            w_rbf = load(rbf_w, [R, D], "w_rbf")
            w_out = load(out_w, [D, 2 * D], "w_out")
            w_self = load(self_w, [D, K * D], "w_self")
            w_msg = load(msg_w, [D, K * D], "w_msg")
            w_upd = load(upd_w, [D, K * 2 * D], "w_upd")
            t_gc2 = load(gc2, [128, R], "gc2")
            t_bias = load(biases, [128, 8], "bias")
            t_msgb = load(msgb_bc, [128, K * D], "msgb")
            t_redm = cpool.tile([128, R * R], f16, tag="redm")
            nc.sync.dma_start(t_redm[:], redmask.ap())

            loop_cm = (tc.For_i(0, loop_reps, 1) if loop_reps
                       else contextlib.nullcontext())
            with loop_cm:
              for _rep in range(n_rep):
                # ============ pairwise sq dist, d, in [j,i] tiles ========
                # per half (8 j-tiles) so RBF can start on half 0 early
                sq = bigpool.tile([128, FREE], f32, tag="sq")
                d = bigpool.tile([128, FREE], f32, tag="d")
                for h in range(2):
                    hs = slice(h * HF, (h + 1) * HF)
                    for jj in range(NJT // 2):
                        jt = h * (NJT // 2) + jj
                        ip = psA.tile([128, CH], f32, tag="ip")
                        nc.tensor.matmul(ip[:],
                                         c_all[:, jt * 128:(jt + 1) * 128],
                                         c_i[:], start=True, stop=True)
                        # sq_part = ip*(-2) + |x_j|^2  (per-partition scalar)
                        nc.vector.tensor_scalar(
                            sq[:, jt * CH:(jt + 1) * CH], ip[:],
                            -2.0, nj[:, jt:jt + 1], ALU.mult, ALU.add)
                        # += |x_i|^2 broadcast along partitions (gpsimd)
                        nc.gpsimd.tensor_tensor(
                            sq[:, jt * CH:(jt + 1) * CH],
                            sq[:, jt * CH:(jt + 1) * CH], ni[:], ALU.add)
                    nc.vector.tensor_scalar_max(sq[:, hs], sq[:, hs], 0.0)
                    nc.scalar.activation(d[:, hs], sq[:, hs], AF.Sqrt)
                    # u' = gamma * sq  (in place over sq)
                    nc.vector.tensor_scalar_mul(sq[:, hs], sq[:, hs], gamma)

                # ============ RBF: Q_r = exp(s_r*d - u' - g*c_r^2) =======
                # STT passes split between DVE and GPSIMD
                gps_rs = set()
                rbf_ps = psB.tile([R, CH], f32, tag="rbf")
                for r in range(R):
                    eng = nc.gpsimd if r in gps_rs else nc.vector
                    for h in range(2):
                        hs = slice(h * HF, (h + 1) * HF)
                        arg = argpool.tile([128, HF], f32, tag="arg")
                        eng.scalar_tensor_tensor(
                            arg[:], d[:, hs], s_r[r], sq[:, hs],
                            ALU.mult, ALU.subtract)
                        qt = argpool.tile([128, HF], f16, tag="q")
                        nc.scalar.activation(qt[:], arg[:], AF.Exp,
                                             bias=t_gc2[:, r:r + 1])
                        redr = t_redm[:, r * R:(r + 1) * R]
                        for jj in range(NJT // 2):
                            nc.tensor.matmul(
                                rbf_ps[:], redr,
                                qt[:, jj * CH:(jj + 1) * CH],
                                start=(r == 0 and h == 0 and jj == 0),
                                stop=(r == R - 1 and h == 1
                                      and jj == NJT // 2 - 1))

                # ============ W = exp(-d) in fp16 ========================
                w_pair = bigpool.tile([128, FREE], f16, tag="wp")
                nc.scalar.activation(w_pair[:], d[:], AF.Exp, scale=-1.0)

                # ============ h_geo, h0 ================================
                rbf_loc = work.tile([R, CH], f32, tag="rbfloc")
                nc.vector.tensor_copy(rbf_loc[:], rbf_ps[:])
                hg_ps = psC.tile([D, CH], f32, tag="mm")
                nc.tensor.matmul(hg_ps[:], w_rbf[:], rbf_loc[:],
                                 start=True, stop=True)
                hg = work.tile([D, CH], f32, tag="hg")
                nc.scalar.activation(hg[:], hg_ps[:], AF.Identity,
                                     bias=t_bias[:, 0:1])
                h0_ps = psC.tile([D, CH], f32, tag="mm")
                nc.tensor.matmul(h0_ps[:], w_out[:, 0:D], hat[:],
                                 start=True, stop=False)
                nc.tensor.matmul(h0_ps[:], w_out[:, D:2 * D], hg[:],
                                 start=False, stop=True)
                h_my = hmyp.tile([D, CH], f32, tag="hmy")
                nc.scalar.activation(h_my[:], h0_ps[:], AF.Identity,
                                     bias=t_bias[:, 1:2])
                if debug_outs:
                    nc.sync.dma_start(dbg_rbf.ap(), rbf_loc[:])
                    nc.sync.dma_start(dbg_h0.ap(), h_my[:])
                    nc.sync.dma_start(dbg_w.ap(), w_pair[:])

                # ============ MP layers ================================
                for k in range(K):
                    # all-gather h_my -> h_full [D, 2048]
                    h_full = hfullp.tile([D, L], f32, tag="hfull")
                    if use_cc:
                        ag_in = dpool.tile([D, CH], f32, tag="agin")
                        ag_out = dpool.tile([4, D, CH], f32, tag="agout")
                        nc.sync.dma_start(ag_in[:], h_my[:])
                        nc.gpsimd.collective_compute(
                            "AllGather", ALU.bypass,
                            replica_groups=[[0, 1, 2, 3], [4, 5, 6, 7]],
                            ins=[ag_in.opt()], outs=[ag_out.opt()],
                        )
                        # dest [d, (q, i)] <- src [q, d, i]
                        nc.sync.dma_start(
                            h_full[:].rearrange("d (q i) -> d q i", q=4),
                            ag_out[:].transpose([1, 0, 2]))
                    else:
                        for q in range(4):
                            nc.vector.tensor_copy(
                                h_full[:, q * CH:(q + 1) * CH], h_my[:])

                    # X[j, dout] tiles (node-major, fp16), msg bias folded
                    x_sb = work.tile([128, NJT, D], f16, tag="xsb")
                    for jt in range(NJT):
                        x_ps = psA.tile([128, D], f32, tag="xps")
                        nc.tensor.matmul(x_ps[:],
                                         h_full[:, jt * 128:(jt + 1) * 128],
                                         w_msg[:, k * D:(k + 1) * D],
                                         start=True, stop=True)
                        nc.vector.tensor_tensor(
                            x_sb[:, jt, :], x_ps[:],
                            t_msgb[:, k * D:(k + 1) * D], ALU.add)
                    # msgT[dout, i] accumulated over j-tiles (fp16 matmuls)
                    msg_ps = psC.tile([D, CH], f32, tag="mm")
                    for jt in range(NJT):
                        nc.tensor.matmul(
                            msg_ps[:], x_sb[:, jt, :],
                            w_pair[:, jt * CH:(jt + 1) * CH],
                            start=(jt == 0), stop=(jt == NJT - 1))
                    msg_sb = work.tile([D, CH], f32, tag="msgsb")
                    nc.scalar.copy(msg_sb[:], msg_ps[:])
                    # selfT
                    self_ps = psC.tile([D, CH], f32, tag="mm")
                    nc.tensor.matmul(self_ps[:],
                                     w_self[:, k * D:(k + 1) * D], h_my[:],
                                     start=True, stop=True)
                    self_sb = work.tile([D, CH], f32, tag="selfsb")
                    nc.scalar.activation(self_sb[:], self_ps[:], AF.Identity,
                                         bias=t_bias[:, 2 + k:3 + k])
                    # update
                    upd_ps = psC.tile([D, CH], f32, tag="mm")
                    nc.tensor.matmul(upd_ps[:],
                                     w_upd[:, (2 * k) * D:(2 * k + 1) * D],
                                     self_sb[:], start=True, stop=False)
                    nc.tensor.matmul(upd_ps[:],
                                     w_upd[:, (2 * k + 1) * D:(2 * k + 2) * D],
                                     msg_sb[:], start=False, stop=True)
                    h_my = hmyp.tile([D, CH], f32, tag="hmy")
                    nc.scalar.activation(h_my[:], upd_ps[:], AF.Identity,
                                         bias=t_bias[:, 5 + k:6 + k])

                nc.sync.dma_start(out_hT.ap(), h_my[:])

    nc.compile()
    return nc


def make_in_maps(inputs):
    """Shard full inputs into per-core input maps (host side)."""
    coords = np.asarray(inputs["coords"], np.float32)        # [B, L, 3]
    Z = np.asarray(inputs["Z"])                              # [B, L]
    atom_emb = np.asarray(inputs["atom_emb"], np.float32)
    gamma = float(np.asarray(inputs["gamma"]))
    centers = np.asarray(inputs["rbf_centers"], np.float64)
    rbf_w = np.asarray(inputs["rbf_w"], np.float32)
    rbf_b = np.asarray(inputs["rbf_b"], np.float32)
    out_w = np.asarray(inputs["out_w"], np.float32)
    out_b = np.asarray(inputs["out_b"], np.float32)
    self_w = np.asarray(inputs["self_w"], np.float32)
    self_b = np.asarray(inputs["self_b"], np.float32)
    msg_w = np.asarray(inputs["msg_w"], np.float32)
    msg_b = np.asarray(inputs["msg_b"], np.float32)
    upd_w = np.asarray(inputs["upd_w"], np.float32)
    upd_b = np.asarray(inputs["upd_b"], np.float32)

    gc2_row = (-gamma * centers.astype(np.float64) ** 2).astype(np.float32)
    gc2 = np.tile(gc2_row[None, :], (128, 1))
    bias_cols = np.stack([rbf_b, out_b, self_b[0], self_b[1], self_b[2],
                          upd_b[0], upd_b[1], upd_b[2]], axis=1)  # [128, 8]
    msgb = np.tile(msg_b.reshape(1, K * D), (128, 1))
    redmask = np.zeros((128, R, R), np.float16)
    for r in range(R):
        redmask[:, r, r] = 1.0 / L
    redmask = redmask.reshape(128, R * R)
    # weight layouts: d_in on partitions, concat along free dim
    self_w_t = np.concatenate([self_w[k] for k in range(K)], axis=1)
    msg_w_t = np.concatenate([msg_w[k] for k in range(K)], axis=1)
    upd_w_t = np.concatenate(
        [upd_w[k, h * D:(h + 1) * D, :] for k in range(K) for h in range(2)],
        axis=1)                                               # [D, K*2*D]
    out_w_t = np.concatenate([out_w[0:D, :], out_w[D:2 * D, :]], axis=1)

    in_maps = []
    for c in range(NCORES):
        b, q = divmod(c, 4)
        sl = slice(q * CH, (q + 1) * CH)
        cb = coords[b]                                       # [L, 3]
        cT = np.zeros((4, L), np.float32)
        cT[:3] = cb.T
        nrm = (cb.astype(np.float64) ** 2).sum(-1).astype(np.float32)
        in_maps.append({
            "cT_all": cT,
            "cT_i": np.ascontiguousarray(cT[:, sl]),
            "nrmj_t": np.ascontiguousarray(nrm.reshape(NJT, 128).T),
            "nrmi_bc": np.ascontiguousarray(np.tile(nrm[sl][None, :],
                                                    (128, 1))),
            "h_atomT": np.ascontiguousarray(atom_emb[Z[b, sl]].T),
            "rbf_w": rbf_w,
            "out_w": np.ascontiguousarray(out_w_t),
            "self_w": np.ascontiguousarray(self_w_t),
            "msg_w": np.ascontiguousarray(msg_w_t),
            "upd_w": np.ascontiguousarray(upd_w_t),
            "gc2": gc2, "biases": np.ascontiguousarray(bias_cols),
            "msgb_bc": np.ascontiguousarray(msgb), "redmask": redmask,
        })
    return gamma, centers, in_maps


def kernel(**inputs):
    from concourse.bass_utils import run_bass_kernel_spmd

    gamma, centers, in_maps = make_in_maps(inputs)
    nc = build_program(gamma, centers)
    res = run_bass_kernel_spmd(nc, in_maps, core_ids=list(range(NCORES)))
    out = np.zeros((B, L, D), np.float32)
    for c in range(NCORES):
        b, q = divmod(c, 4)
        out[b, q * CH:(q + 1) * CH, :] = res.results[c]["out_hT"].T
    return out

